# revision 11
# baseline (speedup 1.0000x reference)
"""Multi-head attention (b=4, n=2048, dim=1024, 16 heads x 64) on 8 Trainium2
NeuronCores.

Sharding: data-parallel over batch (4) x tensor-parallel over head-groups (2).
Each core gets one batch element and 8 heads: it computes its slice of the QKV
projection, full attention for its heads, and a partial output projection.
The host sums the two head-group partials per batch element and adds b_out.

Per-core pipeline (fp32 data; matmul-feeding tiles optionally float32r):
  A1: transpose x [n, dim] -> xT via PE transposes, staged to a DRAM scratch.
  A2: qT = Wq^T x^T and kT = Wk^T x^T (kept transposed, [inner, n] in 128-row
      strips), v = x Wv kept natural [n, inner] augmented with a ones column
      per head (so the PV matmul also produces the softmax denominator).
  B:  per head h, per i-block: S^T tiles = matmul(lhsT=k^T_h j-block,
      rhs=q^T_h) (scores transposed, [j, i]); exp on ScalarE (scale=1/sqrt(dh)
      folded); PV matmul accumulates O_aug^T = v_aug^T @ P^T in PSUM
      ([dh+1, i]; last row = denominator). Tail: reciprocal of the denominator
      row, broadcast across partitions with a K=1 matmul, multiply ->
      normalized O^T.
  C:  y = O @ w_out via lhsT = O^T strips; DMA out natural [n, dim].
"""

import numpy as np

import concourse.bass as bass
import concourse.mybir as mybir
import concourse.tile as tile
from concourse import bacc, bass_utils
from concourse.masks import make_identity

F32 = mybir.dt.float32
AF = mybir.ActivationFunctionType

# Full-problem constants (hardcoded per the harness contract).
B_FULL, N_FULL, DIM_FULL = 4, 2048, 1024
HEADS_FULL, DH = 16, 64
N_CORES = 8
GROUPS = 2                       # head-group (tensor-parallel) factor
HPC = HEADS_FULL // GROUPS       # heads per core = 8
INNER_PC = HPC * DH              # per-core inner dim = 512

# Matmul compute dtype: float32r streams 1 row/cycle (vs 4 for float32) at
# slightly reduced precision. All tiles feeding matmuls carry this dtype
# (producers round into it); numpy float32 maps onto it unchanged.
MM_DT = mybir.dt.float32r


def ts(i, size):
    return slice(i * size, (i + 1) * size)


def emit_core_kernel(nc, tc, x, wqkv, wout, y, *, n, dim, hpc, dh,
                     mm_dt=MM_DT, ib=1024):
    inner = hpc * dh
    KC = dim // 128          # contraction chunks for the qkv projection
    S = inner // 128         # 128-row strips of the per-core inner dim
    JT = n // 128            # key/value j-tiles
    ib = min(ib, n)
    assert n % 512 == 0 and dim % 128 == 0 and inner % 128 == 0
    assert ib % 512 == 0 and n % ib == 0
    scale = float(1.0 / np.sqrt(dh))
    MD = mm_dt

    with (
        tc.tile_pool(name="const", bufs=1) as const_pool,
        tc.tile_pool(name="persist", bufs=1) as persist,
        tc.tile_pool(name="dram", bufs=1, space="DRAM") as dram_pool,
    ):
        # Memset/affine_select can only write plain fp32; constants that feed
        # matmuls are built in fp32 and rounded into mm_dt via DVE copies.
        ident = const_pool.tile([128, 128], F32, name="ident")
        make_identity(nc, ident)
        ones_f32 = const_pool.tile([1, dh], F32, name="ones_f32")
        nc.gpsimd.memset(ones_f32, 1.0)
        ones_sb = const_pool.tile([1, dh], MD, name="ones_sb")
        nc.vector.tensor_copy(ones_sb, ones_f32)
        oneshc = const_pool.tile([128, hpc], F32, name="oneshc")
        nc.gpsimd.memset(oneshc, 1.0)

        xt_dram = dram_pool.tile([KC, 128, n], MD, name="xt_dram")

        # Persistent SBUF tensors.
        qT, kT, oT = [], [], []
        for s in range(S):
            qT.append(persist.tile([128, n], MD, name="qTs", tag=f"qT{s}"))
            kT.append(persist.tile([128, n], MD, name="kTs", tag=f"kT{s}"))
        v_sb = []
        for jt in range(JT):
            vt = persist.tile([128, hpc * (dh + 1)], MD, name="vts",
                              tag=f"v{jt}")
            v_sb.append(vt)
            nc.vector.tensor_copy(
                vt.rearrange("p (h c) -> p h c", c=dh + 1)[:, :, dh:dh + 1],
                oneshc.rearrange("p (h c) -> p h c", c=1))

        # ---- Phase A1: x -> xT (DRAM scratch) via PE transposes ----
        with (
            tc.tile_pool(name="a1_in", bufs=3) as xin_pool,
            tc.tile_pool(name="a1_acc", bufs=2) as xta_pool,
            tc.tile_pool(name="a1_ps", bufs=4, space="PSUM") as psT_pool,
        ):
            xt_acc = {}
            for it in range(n // 128):
                x_in = xin_pool.tile([128, dim], F32, name="x_in")
                nc.sync.dma_start(x_in, x[ts(it, 128), :])
                j = it % 4
                for kc in range(KC):
                    pt = psT_pool.tile([128, 128], F32, name="pt")
                    nc.tensor.transpose(pt, x_in[:, ts(kc, 128)], ident)
                    if j == 0:
                        xt_acc[kc] = xta_pool.tile(
                            [128, 512], MD, name="xt_acc", tag=f"xta{kc}")
                    nc.vector.tensor_copy(xt_acc[kc][:, ts(j, 128)], pt)
                    if j == 3:
                        nc.sync.dma_start(
                            xt_dram[kc, :, (it - 3) * 128:(it + 1) * 128],
                            xt_acc[kc])

        # ---- Phase A2: qT, kT (transposed) and v_aug (natural) ----
        with (
            tc.tile_pool(name="a2_w", bufs=1) as w_pool,
            tc.tile_pool(name="a2_xts", bufs=KC + 2) as xts_pool,
            tc.tile_pool(name="a2_ps", bufs=3, space="PSUM") as psA_pool,
        ):
            w_sb = []
            for kc in range(KC):
                wt = w_pool.tile([128, 3 * inner], MD, name="wt",
                                 tag=f"w{kc}")
                nc.sync.dma_start(wt, wqkv[ts(kc, 128), :])
                w_sb.append(wt)

            for nb in range(n // 512):
                xts = []
                for kc in range(KC):
                    t = xts_pool.tile([128, 512], MD, name="xts")
                    nc.sync.dma_start(t, xt_dram[kc, :, ts(nb, 512)])
                    xts.append(t)
                # qT / kT strips: out[m, i] over this 512-wide i chunk.
                for which, outT in ((0, qT), (1, kT)):
                    for s in range(S):
                        ps = psA_pool.tile([128, 512], F32, name="psA")
                        base = which * inner + s * 128
                        for kc in range(KC):
                            nc.tensor.matmul(
                                ps, w_sb[kc][:, base:base + 128],
                                xts[kc],
                                start=(kc == 0), stop=(kc == KC - 1))
                        nc.vector.tensor_copy(outT[s][:, ts(nb, 512)], ps)
                # v natural: 4 row-tiles of 128 within this chunk.
                for j2 in range(4):
                    it = nb * 4 + j2
                    ps = psA_pool.tile([128, inner], F32, name="psAv",
                                       tag="psAv")
                    for kc in range(KC):
                        nc.tensor.matmul(
                            ps, xts[kc][:, ts(j2, 128)],
                            w_sb[kc][:, 2 * inner:3 * inner],
                            start=(kc == 0), stop=(kc == KC - 1))
                    nc.vector.tensor_copy(
                        v_sb[it].rearrange(
                            "p (h c) -> p h c", c=dh + 1)[:, :, 0:dh],
                        ps.rearrange("p (h c) -> p h c", c=dh))

        # ---- Phase B: attention per head ----
        for s in range(S):
            oT.append(persist.tile([128, n], MD, name="oTs", tag=f"oT{s}"))
        with (
            tc.tile_pool(name="b_psS", bufs=2, space="PSUM") as psS_pool,
            tc.tile_pool(name="b_psO", bufs=1, space="PSUM") as psO_pool,
            tc.tile_pool(name="b_psB", bufs=1, space="PSUM") as psB_pool,
            tc.tile_pool(name="b_pexp", bufs=3) as pexp_pool,
            tc.tile_pool(name="b_tail", bufs=2) as tail_pool,
        ):
            for h in range(hpc):
                s_, r_ = divmod(h * dh, 128)
                kTh = kT[s_][r_:r_ + dh, :]
                qTh = qT[s_][r_:r_ + dh, :]
                vcol = slice(h * (dh + 1), (h + 1) * (dh + 1))
                for ibx in range(n // ib):
                    po = psO_pool.tile([dh + 1, ib], F32, name="po")

                    def emit_pv(pexp, jt):
                        for c in range(ib // 512):
                            nc.tensor.matmul(
                                po[:, ts(c, 512)], v_sb[jt][:, vcol],
                                pexp[:, ts(c, 512)],
                                start=(jt == 0), stop=(jt == JT - 1))

                    # software-pipelined: S(jt+1) is emitted before PV(jt)
                    # so the in-order PE never stalls on ScalarE's exp.
                    pend = None
                    for jt in range(JT):
                        psS = psS_pool.tile([128, ib], F32, name="psS")
                        for c in range(ib // 512):
                            nc.tensor.matmul(
                                psS[:, ts(c, 512)], kTh[:, ts(jt, 128)],
                                qTh[:, ibx * ib + c * 512:
                                    ibx * ib + c * 512 + 512],
                                start=True, stop=True)
                        pexp = pexp_pool.tile([128, ib], MD, name="pexp")
                        nc.scalar.activation(pexp, psS, AF.Exp, scale=scale)
                        if pend is not None:
                            emit_pv(*pend)
                        pend = (pexp, jt)
                    emit_pv(*pend)

                    # tail: normalize rows 0..dh-1 by the denominator row.
                    recip_f = tail_pool.tile([1, ib], F32, name="recip_f")
                    nc.vector.reciprocal(recip_f, po[dh:dh + 1, :])
                    recip = tail_pool.tile([1, ib], MD, name="recip")
                    nc.vector.tensor_copy(recip, recip_f)
                    pb = psB_pool.tile([dh, ib], F32, name="pb")
                    for c in range(ib // 512):
                        nc.tensor.matmul(pb[:, ts(c, 512)], ones_sb,
                                         recip[:, ts(c, 512)],
                                         start=True, stop=True)
                    bc = tail_pool.tile([dh, ib], F32, name="bc")
                    nc.vector.tensor_copy(bc, pb)
                    nc.vector.tensor_mul(
                        oT[s_][r_:r_ + dh, ts(ibx, ib)], po[0:dh, :], bc)

        # ---- Phase C: y = O @ w_out ----
        with (
            tc.tile_pool(name="c_w", bufs=1) as wout_pool,
            tc.tile_pool(name="c_y", bufs=3) as y_pool,
            tc.tile_pool(name="c_ps", bufs=3, space="PSUM") as psC_pool,
        ):
            wout_sb = []
            for t in range(S):
                wo = wout_pool.tile([128, dim], MD, name="wo", tag=f"wo{t}")
                nc.sync.dma_start(wo, wout[ts(t, 128), :])
                wout_sb.append(wo)
            fc = min(512, dim)
            for it in range(n // 128):
                ysb = y_pool.tile([128, dim], F32, name="ysb")
                for c in range(dim // fc):
                    ps = psC_pool.tile([128, fc], F32, name="psC")
                    for t in range(S):
                        nc.tensor.matmul(
                            ps, oT[t][:, ts(it, 128)],
                            wout_sb[t][:, ts(c, fc)],
                            start=(t == 0), stop=(t == S - 1))
                    nc.vector.tensor_copy(ysb[:, ts(c, fc)], ps)
                nc.sync.dma_start(y[ts(it, 128), :], ysb)


_BUILD_CACHE = {}


def build_nc(n=N_FULL, dim=DIM_FULL, hpc=HPC, dh=DH, mm_dt=MM_DT, ib=1024):
    key = (n, dim, hpc, dh, str(mm_dt), ib)
    if key in _BUILD_CACHE:
        return _BUILD_CACHE[key]
    inner = hpc * dh
    nc = bacc.Bacc("TRN2", target_bir_lowering=False, debug=False)
    x = nc.dram_tensor("x", [n, dim], F32, kind="ExternalInput").ap()
    wqkv = nc.dram_tensor("w_qkv", [dim, 3 * inner], mm_dt,
                          kind="ExternalInput").ap()
    wout = nc.dram_tensor("w_out", [inner, dim], mm_dt,
                          kind="ExternalInput").ap()
    y = nc.dram_tensor("y", [n, dim], F32, kind="ExternalOutput").ap()
    with tile.TileContext(nc) as tc:
        with nc.allow_low_precision(
                reason="float32r is 4-byte; PSUM accumulation stays fp32"):
            emit_core_kernel(nc, tc, x, wqkv, wout, y, n=n, dim=dim, hpc=hpc,
                             dh=dh, mm_dt=mm_dt, ib=ib)
    nc.compile()
    _BUILD_CACHE[key] = nc
    return nc


def make_in_maps(x, w_qkv, w_out):
    """Shard full inputs into the 8 per-core input maps."""
    x = np.asarray(x, dtype=np.float32)
    w_qkv = np.asarray(w_qkv, dtype=np.float32)
    w_out = np.asarray(w_out, dtype=np.float32)
    qk_off = HEADS_FULL * DH          # 1024: start of K block in w_qkv
    in_maps = []
    for c in range(N_CORES):
        b, g = divmod(c, GROUPS)
        cols = ts(g, INNER_PC)
        wq = w_qkv[:, cols]
        wk = w_qkv[:, qk_off + g * INNER_PC: qk_off + (g + 1) * INNER_PC]
        wv = w_qkv[:, 2 * qk_off + g * INNER_PC: 2 * qk_off + (g + 1) * INNER_PC]
        in_maps.append({
            "x": np.ascontiguousarray(x[b]),
            "w_qkv": np.ascontiguousarray(np.concatenate([wq, wk, wv], axis=1)),
            "w_out": np.ascontiguousarray(w_out[cols, :]),
        })
    return in_maps


def kernel(x, w_qkv, w_out, b_out, trace=False):
    b_out = np.asarray(b_out, dtype=np.float32)
    nc = build_nc()
    in_maps = make_in_maps(x, w_qkv, w_out)
    res = bass_utils.run_bass_kernel_spmd(
        nc, in_maps, core_ids=list(range(N_CORES)), trace=trace)
    ys = [r["y"] for r in res.results]
    out = np.empty((B_FULL, N_FULL, DIM_FULL), dtype=np.float32)
    for b in range(B_FULL):
        out[b] = ys[GROUPS * b] + ys[GROUPS * b + 1] + b_out[None, :]
    if trace:
        kernel.last_result = res
    return out
